# revision 1
# baseline (speedup 1.0000x reference)
"""CategorySpecificLinear TRN2 kernel.

out[b] = x[b] @ W[cat_ids[b]] + bias[cat_ids[b]]
  x: [64, 512, 1024] f32, W: [32, 1024, 4096] f32, b: [32, 4096] f32 -> out [64, 512, 4096] f32

Strategy: data-parallel over batch (8 batches per core on 8 cores). The
category gather and the x transpose are done on the host (cat_ids are known
at launch), so each core receives its 8 per-batch weight matrices directly.
Matmuls run in fp16 (fp32 PSUM accumulation): same PE throughput as bf16 on
TRN2 but ~8x better accuracy (~3e-4 rel), 4x faster than native fp32.
"""
import numpy as np

B_TOTAL = 64
N_CORES = 8
B = B_TOTAL // N_CORES  # batches per core
S = 512    # seq
K = 1024   # input_dim
H = 4096   # hidden_dim
P = 128
KT = K // P   # 8 k-tiles
MT = S // P   # 4 m-tiles
NW = 512      # hidden tile width (one PSUM bank of f32)
NT = H // NW  # 8 n-tiles

_NC = None


def _build_nc():
    global _NC
    if _NC is not None:
        return _NC

    import concourse.mybir as mybir
    import concourse.tile as tile
    from concourse import bacc

    f16 = mybir.dt.float16
    f32 = mybir.dt.float32

    nc = bacc.Bacc("TRN2", target_bir_lowering=False, debug=False, num_devices=N_CORES)
    xt = nc.dram_tensor("xt", [B, K, S], f16, kind="ExternalInput").ap()
    w = nc.dram_tensor("w", [B, K, H], f16, kind="ExternalInput").ap()
    bias = nc.dram_tensor("bias", [B, H], f32, kind="ExternalInput").ap()
    out = nc.dram_tensor("out", [B, S, H], f32, kind="ExternalOutput").ap()

    with tile.TileContext(nc) as tc:
        with (
            tc.tile_pool(name="xtp", bufs=2) as xtp,
            tc.tile_pool(name="wp", bufs=3) as wp,
            tc.tile_pool(name="bp", bufs=2) as bp,
            tc.tile_pool(name="op", bufs=4) as op,
            tc.tile_pool(name="ps", bufs=4, space="PSUM") as ps,
        ):
            for b_i in range(B):
                xt_sb = xtp.tile([P, KT, S], f16, tag="xt")
                nc.sync.dma_start(xt_sb[:], xt[b_i].rearrange("(ko p) s -> p ko s", p=P))
                bias_row = bp.tile([1, H], f32, tag="bias_row")
                nc.sync.dma_start(bias_row[:], bias[b_i][None, :])
                bias_bc = bp.tile([P, H], f32, tag="bias_bc")
                nc.gpsimd.partition_broadcast(bias_bc[:], bias_row[:])
                for n_i in range(NT):
                    w_sb = wp.tile([P, KT, NW], f16, tag="w")
                    nc.sync.dma_start(
                        w_sb[:],
                        w[b_i, :, n_i * NW : (n_i + 1) * NW].rearrange(
                            "(ko p) n -> p ko n", p=P
                        ),
                    )
                    for m_i in range(MT):
                        pt = ps.tile([P, NW], f32, tag="psum")
                        for k_i in range(KT):
                            nc.tensor.matmul(
                                pt[:],
                                xt_sb[:, k_i, m_i * P : (m_i + 1) * P],
                                w_sb[:, k_i, :],
                                start=(k_i == 0),
                                stop=(k_i == KT - 1),
                            )
                        ot = op.tile([P, NW], f32, tag="out")
                        nc.vector.tensor_add(
                            ot[:], pt[:], bias_bc[:, n_i * NW : (n_i + 1) * NW]
                        )
                        nc.sync.dma_start(
                            out[b_i, m_i * P : (m_i + 1) * P, n_i * NW : (n_i + 1) * NW],
                            ot[:],
                        )
    nc.compile()
    _NC = nc
    return nc


def kernel(x, cat_ids, W, b):
    from concourse.bass_utils import run_bass_kernel_spmd

    x = np.asarray(x)
    cat_ids = np.asarray(cat_ids).astype(np.int64)
    W = np.asarray(W)
    b = np.asarray(b)

    nc = _build_nc()

    # Host-side prep: fp16 conversion, per-batch category gather, x transpose.
    W16 = W.astype(np.float16)                      # [32, K, H]
    Wg = W16[cat_ids]                               # [64, K, H]
    x16 = x.astype(np.float16)                      # [64, S, K]
    xt16 = np.ascontiguousarray(x16.transpose(0, 2, 1))  # [64, K, S]
    bg = b[cat_ids].astype(np.float32)              # [64, H]

    in_maps = []
    for c in range(N_CORES):
        sl = slice(B * c, B * (c + 1))
        in_maps.append(
            {
                "xt": np.ascontiguousarray(xt16[sl]),
                "w": np.ascontiguousarray(Wg[sl]),
                "bias": np.ascontiguousarray(bg[sl]),
            }
        )

    res = run_bass_kernel_spmd(nc, in_maps, core_ids=list(range(N_CORES)))
    out = np.concatenate([r["out"] for r in res.results], axis=0)
    return out.astype(np.float32, copy=False)
